# revision 1
# baseline (speedup 1.0000x reference)
"""GroupAwareContrastiveLoss Trainium2 kernel.

Strategy (sharding_hint: shard rows i across 8 cores, replicate codebook):
  - Host normalizes the codebook once (zn = z/||z||, bf16) and ships each
    core a column-rotated copy of zn^T so that every core's own 1024 rows
    land in local columns [0, 1024) -- this makes the "diagonal"
    (range-mask / j==i) col-blocks identical across cores, keeping the
    program SPMD while masks stay data-driven.
  - Device computes cos = zn_i . zn_j via one bf16 matmul per
    (row-tile, col-block), then:
      neg part: relu(c-0.1)^2 + relu(-c-0.1)^2 summed over ALL j (ACT relu
        + DVE tensor_tensor_reduce), minus a masked correction over the
        in-range/j==i entries (only on active blocks).
      pos part (active blocks only): d2 = sq_i + sq_j - 2*nrm_i*nrm_j*c,
        D = sqrt(max(d2,0)), relu(D-0.5)^2 masked-summed.
  - Per-row sums return to host; host does the O(M) counting, division,
    valid-masking and the final scalar mean (plus the exact j==i ortho
    constant 0.81 that the device masked out).
"""

import os
import sys
import numpy as np

if "/opt/trn_rl_repo" not in sys.path:
    sys.path.insert(0, "/opt/trn_rl_repo")

from contextlib import ExitStack

import concourse.bass as bass
import concourse.bacc as bacc
import concourse.mybir as mybir
from concourse import tile
from concourse.alu_op_type import AluOpType as ALU
from concourse.bass_utils import run_bass_kernel_spmd

N = 8192          # total codebook rows (= cols of the cos matrix)
D = 1024          # feature dim
NCORES = 8
T = 8             # 128-row tiles per core (8*128 = 1024 rows/core)
BLK = 512         # col-block width (one PSUM bank of fp32)
NBLK = N // BLK   # 16
KCH = D // 128    # 8 contraction chunks
ROWS_PER_CORE = T * 128

M_POS = 0.5
M_NEG_SIM = 0.1
LAM_NEG = 1.0

FP32 = mybir.dt.float32
BF16 = mybir.dt.bfloat16
AF = mybir.ActivationFunctionType

# program cache: signature -> bass.Bass
_programs = {}

# filled by the most recent kernel() call (for test harnesses)
last_exec_time_ns = None
last_result = None


def _build_program(active_sig, use_i2):
    """active_sig: tuple over t of sorted tuple of range-active col blocks."""
    dma_gp = bool(os.environ.get("KDMA_GPSIMD"))
    nc = bacc.Bacc(
        "TRN2",
        target_bir_lowering=False,
        debug=False,
        num_devices=int(os.environ.get("KNDEV", "1")),
    )

    znt = nc.declare_dram_parameter("znt", [D, N], BF16, isOutput=False)
    bc = nc.declare_dram_parameter("bc", [2, 128, N], FP32, isOutput=False)
    scal = nc.declare_dram_parameter("scal", [T, 128, 12], FP32, isOutput=False)
    iota_d = nc.declare_dram_parameter("iota", [128, BLK], FP32, isOutput=False)
    sums = nc.declare_dram_parameter("sums", [T, 128, 2], FP32, isOutput=True)

    dma = nc.gpsimd.dma_start if dma_gp else nc.sync.dma_start

    with tile.TileContext(nc) as tc, ExitStack() as ctx:
        res_pool = ctx.enter_context(tc.tile_pool(name="res", bufs=1))
        rhs_pool = ctx.enter_context(tc.tile_pool(name="rhs", bufs=2))
        psum_pool = ctx.enter_context(
            tc.tile_pool(name="psum", bufs=4, space="PSUM")
        )
        hot_pool = ctx.enter_context(tc.tile_pool(name="hot", bufs=4))
        diag_pool = ctx.enter_context(tc.tile_pool(name="diag", bufs=2))
        bc_pool = ctx.enter_context(tc.tile_pool(name="bcp", bufs=3))

        # ---- resident loads ----
        lhs = []
        for kk in range(KCH):
            tl = res_pool.tile([128, ROWS_PER_CORE], BF16, tag=f"lhs{kk}", name=f"lhs{kk}")
            dma(tl[:], znt[kk * 128:(kk + 1) * 128, 0:ROWS_PER_CORE])
            lhs.append(tl)

        iota_sb = res_pool.tile([128, BLK], FP32, tag="iota", name="iota_sb")
        dma(iota_sb[:], iota_d[:])

        scal_sb, negfull, negcorr, posacc = [], [], [], []
        for t in range(T):
            st = res_pool.tile([128, 12], FP32, tag=f"scal{t}", name=f"scal{t}")
            dma(st[:], scal[t])
            scal_sb.append(st)
            negfull.append(res_pool.tile([128, 2 * NBLK], FP32, tag=f"nf{t}", name=f"nf{t}"))
            negcorr.append(res_pool.tile([128, NBLK], FP32, tag=f"ncr{t}", name=f"ncr{t}"))
            posacc.append(res_pool.tile([128, NBLK], FP32, tag=f"pa{t}", name=f"pa{t}"))

        ncorr_col = [0] * T
        pos_col = [0] * T

        for b in range(NBLK):
            rhs = []
            for kk in range(KCH):
                tr = rhs_pool.tile([128, BLK], BF16, tag=f"rhs{kk}", name=f"rhs{kk}")
                dma(
                    tr[:], znt[kk * 128:(kk + 1) * 128, b * BLK:(b + 1) * BLK]
                )
                rhs.append(tr)

            # local-coordinate iota for this block, shared across row-tiles
            iota_b = None
            # bcast tiles shared across row-tiles of this block
            nrm_bc = sq_bc = None

            for t in range(T):
                C = psum_pool.tile([128, BLK], FP32, tag="C", name="C")
                for kk in range(KCH):
                    nc.tensor.matmul(
                        C[:],
                        lhs[kk][:, t * 128:(t + 1) * 128],
                        rhs[kk][:],
                        start=(kk == 0),
                        stop=(kk == KCH - 1),
                    )

                # hot path: full-row sum of relu(c-0.1)^2 + relu(-c-0.1)^2
                P1 = hot_pool.tile([128, BLK], BF16, tag="P1", name="P1")
                nc.scalar.activation(P1[:], C[:], AF.Relu, bias=scal_sb[t][:, 8:9], scale=1.0)
                N1 = hot_pool.tile([128, BLK], BF16, tag="N1", name="N1")
                nc.scalar.activation(N1[:], C[:], AF.Relu, bias=scal_sb[t][:, 8:9], scale=-1.0)
                s1t = hot_pool.tile([128, BLK], BF16, tag="s1t", name="s1t")
                nc.vector.scalar_tensor_tensor(
                    out=s1t[:], in0=P1[:], in1=P1[:],  scalar=1.0,
                    op0=ALU.mult, op1=ALU.mult,
                    accum_out=negfull[t][:, 2 * b:2 * b + 1],
                )
                s2t = hot_pool.tile([128, BLK], BF16, tag="s2t", name="s2t")
                nc.vector.scalar_tensor_tensor(
                    out=s2t[:], in0=N1[:], in1=N1[:],  scalar=1.0,
                    op0=ALU.mult, op1=ALU.mult,
                    accum_out=negfull[t][:, 2 * b + 1:2 * b + 2],
                )

                eq_here = (b == t // 4)
                rng = b in active_sig[t]
                if os.environ.get("KDIAG_OFF"):
                    continue
                if not (eq_here or rng):
                    continue

                st = scal_sb[t]
                s1c, e1c = st[:, 0:1], st[:, 1:2]
                s2c, e2c = st[:, 2:3], st[:, 3:4]
                ilc = st[:, 4:5]
                nrmc, sqc, m2nc = st[:, 5:6], st[:, 6:7], st[:, 7:8]

                if iota_b is None:
                    iota_b = diag_pool.tile([128, BLK], FP32, tag="iotab", name="iotab")
                    nc.vector.tensor_scalar(
                        iota_b[:], iota_sb[:], float(b * BLK), None, op0=ALU.add
                    )

                # in-range mask m (local coords), possibly two intervals
                m = None
                if rng:
                    m1 = diag_pool.tile([128, BLK], FP32, tag="m1", name="m1")
                    nc.vector.tensor_scalar(m1[:], iota_b[:], s1c, None, op0=ALU.is_ge)
                    m_a = diag_pool.tile([128, BLK], FP32, tag="ma", name="ma")
                    nc.vector.scalar_tensor_tensor(
                        m_a[:], in0=iota_b[:], scalar=e1c, in1=m1[:],
                        op0=ALU.is_le, op1=ALU.mult,
                    )
                    if use_i2:
                        mb1 = diag_pool.tile([128, BLK], FP32, tag="mb1", name="mb1")
                        nc.vector.tensor_scalar(
                            mb1[:], iota_b[:], s2c, None, op0=ALU.is_ge
                        )
                        m_b = diag_pool.tile([128, BLK], FP32, tag="mb", name="mb")
                        nc.vector.scalar_tensor_tensor(
                            m_b[:], in0=iota_b[:], scalar=e2c, in1=mb1[:],
                            op0=ALU.is_le, op1=ALU.mult,
                        )
                        m = diag_pool.tile([128, BLK], FP32, tag="m", name="m")
                        nc.vector.tensor_tensor(m[:], m_a[:], m_b[:], op=ALU.max)
                    else:
                        m = m_a

                # m2 = mask of entries to REMOVE from the neg sum
                #    = in_range | (j == i); mpos = in_range & (j != i)
                if rng and eq_here:
                    m2 = diag_pool.tile([128, BLK], FP32, tag="m2", name="m2")
                    nc.vector.scalar_tensor_tensor(
                        m2[:], in0=iota_b[:], scalar=ilc, in1=m[:],
                        op0=ALU.is_equal, op1=ALU.max,
                    )
                    mpos = diag_pool.tile([128, BLK], FP32, tag="mpos", name="mpos")
                    nc.vector.scalar_tensor_tensor(
                        mpos[:], in0=iota_b[:], scalar=ilc, in1=m[:],
                        op0=ALU.not_equal, op1=ALU.mult,
                    )
                elif eq_here:
                    m2 = diag_pool.tile([128, BLK], FP32, tag="m2", name="m2")
                    nc.vector.tensor_scalar(
                        m2[:], iota_b[:], ilc, None, op0=ALU.is_equal
                    )
                    mpos = None
                else:
                    m2 = m
                    mpos = m

                # neg correction: sum over m2 of (P1^2 + N1^2)
                nterm = diag_pool.tile([128, BLK], FP32, tag="nterm", name="nterm")
                nc.vector.tensor_tensor(nterm[:], s1t[:], s2t[:], op=ALU.add)
                scrc = diag_pool.tile([128, BLK], FP32, tag="scrc", name="scrc")
                nc.vector.scalar_tensor_tensor(
                    out=scrc[:], in0=nterm[:], in1=m2[:],  scalar=1.0,
                    op0=ALU.mult, op1=ALU.mult,
                    accum_out=negcorr[t][:, ncorr_col[t]:ncorr_col[t] + 1],
                )
                ncorr_col[t] += 1

                # pos chain
                if rng:
                    if nrm_bc is None:
                        nrm_bc = bc_pool.tile([128, BLK], FP32, tag="nbc", name="nbc")
                        dma(
                            nrm_bc[:], bc[0, :, b * BLK:(b + 1) * BLK]
                        )
                        sq_bc = bc_pool.tile([128, BLK], FP32, tag="sbc", name="sbc")
                        dma(
                            sq_bc[:], bc[1, :, b * BLK:(b + 1) * BLK]
                        )
                    u = diag_pool.tile([128, BLK], FP32, tag="u", name="u")
                    nc.vector.scalar_tensor_tensor(
                        u[:], in0=C[:], scalar=m2nc, in1=nrm_bc[:],
                        op0=ALU.mult, op1=ALU.mult,
                    )
                    w = diag_pool.tile([128, BLK], FP32, tag="w", name="w")
                    nc.vector.scalar_tensor_tensor(
                        w[:], in0=u[:], scalar=sqc, in1=sq_bc[:],
                        op0=ALU.add, op1=ALU.add,
                    )
                    w2 = diag_pool.tile([128, BLK], FP32, tag="w2", name="w2")
                    nc.vector.tensor_scalar(w2[:], w[:], 0.0, None, op0=ALU.max)
                    Dt = diag_pool.tile([128, BLK], FP32, tag="Dt", name="Dt")
                    nc.scalar.activation(Dt[:], w2[:], AF.Sqrt, bias=st[:, 10:11])
                    R = diag_pool.tile([128, BLK], FP32, tag="R", name="R")
                    nc.scalar.activation(R[:], Dt[:], AF.Relu, bias=st[:, 9:10])
                    R2 = diag_pool.tile([128, BLK], FP32, tag="R2", name="R2")
                    nc.scalar.activation(R2[:], R[:], AF.Square, bias=st[:, 10:11])
                    scrp = diag_pool.tile([128, BLK], FP32, tag="scrp", name="scrp")
                    nc.vector.scalar_tensor_tensor(
                        out=scrp[:], in0=R2[:], in1=mpos[:], 
                        scalar=1.0, op0=ALU.mult, op1=ALU.mult,
                        accum_out=posacc[t][:, pos_col[t]:pos_col[t] + 1],
                    )
                    pos_col[t] += 1

        # ---- finalize per row-tile ----
        for t in range(T):
            res = res_pool.tile([128, 2], FP32, tag=f"out{t}", name=f"out{t}")
            if pos_col[t] > 0:
                nc.vector.tensor_reduce(
                    res[:, 0:1], posacc[t][:, 0:pos_col[t]],
                    axis=mybir.AxisListType.X, op=ALU.add,
                )
            else:
                nc.vector.memset(res[:, 0:1], 0.0)
            nF = res_pool.tile([128, 1], FP32, tag=f"nF{t}", name=f"nF{t}")
            nc.vector.tensor_reduce(
                nF[:], negfull[t][:], axis=mybir.AxisListType.X, op=ALU.add
            )
            if ncorr_col[t] > 0:
                nC = res_pool.tile([128, 1], FP32, tag=f"nC{t}", name=f"nC{t}")
                nc.vector.tensor_reduce(
                    nC[:], negcorr[t][:, 0:ncorr_col[t]],
                    axis=mybir.AxisListType.X, op=ALU.add,
                )
                nc.vector.tensor_sub(res[:, 1:2], nF[:], nC[:])
            else:
                nc.vector.tensor_copy(res[:, 1:2], nF[:])
            dma(sums[t], res[:])

    nc.compile()
    return nc


def _prepare_inputs(codebook, starts, ends):
    """Build the per-core input maps + the active-block signature."""
    import ml_dtypes

    cb = np.asarray(codebook, dtype=np.float32)
    s_arr = np.asarray(starts).astype(np.int64)
    e_arr = np.asarray(ends).astype(np.int64)

    sq64 = np.sum(cb.astype(np.float64) ** 2, axis=-1)
    nrm = np.sqrt(sq64).astype(np.float32)
    sq = sq64.astype(np.float32)
    zn = (cb / nrm[:, None]).astype(ml_dtypes.bfloat16)
    znt = np.ascontiguousarray(zn.T)  # (D, N)

    iota_np = np.ascontiguousarray(
        np.broadcast_to(np.arange(BLK, dtype=np.float32), (128, BLK))
    )

    # clipped/validated ranges in global coords
    s_cl = np.maximum(s_arr, 0)
    e_cl = np.minimum(e_arr, N - 1)
    nonempty = s_cl <= e_cl

    in_maps = []
    active = [set() for _ in range(T)]
    use_i2 = False
    for c in range(NCORES):
        off = c * ROWS_PER_CORE
        znt_c = np.ascontiguousarray(np.roll(znt, -off, axis=1))
        bc_c = np.ascontiguousarray(
            np.stack(
                [
                    np.broadcast_to(np.roll(nrm, -off), (128, N)),
                    np.broadcast_to(np.roll(sq, -off), (128, N)),
                ]
            ).astype(np.float32)
        )

        r = off + np.arange(ROWS_PER_CORE)  # global row ids
        sL = (s_cl[r] - off) % N
        eL = (e_cl[r] - off) % N
        wrap = nonempty[r] & (sL > eL)
        use_i2 = use_i2 or bool(wrap.any())

        # interval 1 / interval 2 in local coords; empty -> (2, 1)
        i1s = np.where(nonempty[r], np.where(wrap, 0, sL), 2).astype(np.float64)
        i1e = np.where(nonempty[r], eL, 1).astype(np.float64)
        i2s = np.where(wrap, sL, 2).astype(np.float64)
        i2e = np.where(wrap, N - 1, 1).astype(np.float64)

        scal_c = np.zeros((T, 128, 12), dtype=np.float32)
        flat = scal_c.reshape(ROWS_PER_CORE, 12)
        flat[:, 8] = -M_NEG_SIM
        flat[:, 9] = -M_POS
        flat[:, 10] = 0.0
        flat[:, 0] = i1s
        flat[:, 1] = i1e
        flat[:, 2] = i2s
        flat[:, 3] = i2e
        flat[:, 4] = np.arange(ROWS_PER_CORE)  # local row index
        flat[:, 5] = nrm[r]
        flat[:, 6] = sq[r]
        flat[:, 7] = -2.0 * nrm[r]

        # active col-blocks per local row-tile (union across cores)
        for t in range(T):
            rt = slice(t * 128, (t + 1) * 128)
            for ss, ee, mask in (
                (i1s[rt], i1e[rt], None),
                (i2s[rt], i2e[rt], None),
            ):
                ok = ss <= ee
                if not ok.any():
                    continue
                b_lo = (ss[ok].astype(np.int64)) // BLK
                b_hi = (ee[ok].astype(np.int64)) // BLK
                for lo, hi in zip(b_lo, b_hi):
                    for bb in range(int(lo), int(hi) + 1):
                        active[t].add(bb)

        in_maps.append(
            {"znt": znt_c, "bc": bc_c, "scal": scal_c, "iota": iota_np}
        )

    sig = tuple(tuple(sorted(a)) for a in active)
    return in_maps, sig, use_i2


def _host_finalize(pos_dev, neg_dev, starts, ends, M):
    """pos_dev/neg_dev: (N,) per-row masked sums from the device."""
    s_arr = np.asarray(starts).astype(np.int64)[:M]
    e_arr = np.asarray(ends).astype(np.int64)[:M]
    i_arr = np.arange(M, dtype=np.int64)

    lo = np.maximum(s_arr, 0)
    hi = np.minimum(e_arr, N - 1)
    cnt_in = np.maximum(0, hi - lo + 1)
    in_i = ((i_arr >= s_arr) & (i_arr <= e_arr)).astype(np.int64)
    pos_cnt = cnt_in - in_i
    neg_cnt = N - cnt_in + in_i

    diag_term = np.float32(1.0 - M_NEG_SIM) ** 2  # exact j==i ortho entry
    pos_sum = pos_dev[:M].astype(np.float64)
    neg_sum = neg_dev[:M].astype(np.float64) + float(diag_term)

    pos_pull = pos_sum / np.maximum(pos_cnt, 1)
    ortho = neg_sum / np.maximum(neg_cnt, 1)
    valid = (pos_cnt > 0) & (neg_cnt > 0)
    per_row = np.where(valid, pos_pull + LAM_NEG * ortho, 0.0)
    cnt = int(valid.sum())
    total = per_row.sum()
    if cnt > 0:
        return np.float32(total / cnt)
    return np.float32(0.0)


# cached jitted executables: program-key -> dict with callable + metadata
_exec_cache = {}
_last_bench = None  # (sharded, concat_in_dev, zero_shapes, out_names, out_avals)


def _get_exec(nc, key):
    import jax
    from jax.sharding import Mesh, PartitionSpec
    from jax.experimental.shard_map import shard_map
    from concourse import bass2jax
    from concourse.bass2jax import _bass_exec_p

    if key in _exec_cache:
        return _exec_cache[key]

    bass2jax.install_neuronx_cc_hook()

    in_names, out_names, out_avals, zero_shapes = [], [], [], []
    for alloc in nc.m.functions[0].allocations:
        if not isinstance(alloc, mybir.MemoryLocationSet):
            continue
        name = alloc.memorylocations[0].name
        if alloc.kind == "ExternalInput":
            in_names.append(name)
        elif alloc.kind == "ExternalOutput":
            out_names.append(name)
            shape = tuple(alloc.tensor_shape)
            dtype = mybir.dt.np(alloc.dtype)
            out_avals.append(jax.core.ShapedArray(shape, dtype))
            zero_shapes.append((shape, dtype))
    part_name = (
        nc.partition_id_tensor.name if nc.partition_id_tensor else None
    )
    if part_name is not None and part_name in in_names:
        in_names.remove(part_name)
    n_params = len(in_names)
    all_names = in_names + out_names
    if part_name is not None:
        all_names = all_names + [part_name]
    donate = tuple(range(n_params, n_params + len(out_names)))

    def _body(*args):
        operands = list(args)
        if part_name is not None:
            operands.append(bass2jax.partition_id_tensor())
        outs = _bass_exec_p.bind(
            *operands,
            out_avals=tuple(out_avals),
            in_names=tuple(all_names),
            out_names=tuple(out_names),
            lowering_input_output_aliases=(),
            sim_require_finite=True,
            sim_require_nnan=True,
            nc=nc,
        )
        return tuple(outs)

    devices = jax.devices()[:NCORES]
    mesh = Mesh(np.asarray(devices), ("core",))
    in_specs = (PartitionSpec("core"),) * (n_params + len(out_names))
    out_specs = (PartitionSpec("core"),) * len(out_names)
    sharded = jax.jit(
        shard_map(_body, mesh=mesh, in_specs=in_specs, out_specs=out_specs,
                  check_rep=False),
        donate_argnums=donate,
        keep_unused=True,
    )
    info = {
        "mesh": mesh,
        "sharded": sharded,
        "in_names": in_names,
        "out_names": out_names,
        "out_avals": out_avals,
        "zero_shapes": zero_shapes,
        "n_params": n_params,
    }
    _exec_cache[key] = info
    return info


def _run_programs(nc, key, in_maps):
    """Execute the SPMD program on 8 cores; returns list of out dicts."""
    global _last_bench
    import jax

    info = _get_exec(nc, key)
    concat_in = [
        np.concatenate([np.asarray(m[name]) for m in in_maps], axis=0)
        for name in info["in_names"]
    ]
    from jax.sharding import NamedSharding, PartitionSpec
    shd = NamedSharding(info["mesh"], PartitionSpec("core"))
    concat_in_dev = jax.block_until_ready(
        [jax.device_put(a, shd) for a in concat_in]
    )
    zeros = [
        np.zeros((NCORES * s[0], *s[1:]), d) for (s, d) in info["zero_shapes"]
    ]
    out_arrs = jax.block_until_ready(info["sharded"](*concat_in_dev, *zeros))
    _last_bench = (info, concat_in_dev)
    results = [
        {
            name: np.asarray(out_arrs[i]).reshape(
                NCORES, *info["out_avals"][i].shape
            )[c]
            for i, name in enumerate(info["out_names"])
        }
        for c in range(NCORES)
    ]
    return results


def benchmark_last(iters=20):
    """Re-run the last executable; returns per-iteration seconds (median)."""
    import time
    import jax

    info, concat_in_dev = _last_bench
    times = []
    for _ in range(iters):
        zeros = [
            np.zeros((NCORES * s[0], *s[1:]), d)
            for (s, d) in info["zero_shapes"]
        ]
        t0 = time.perf_counter()
        jax.block_until_ready(info["sharded"](*concat_in_dev, *zeros))
        times.append(time.perf_counter() - t0)
    times.sort()
    return times[len(times) // 2]


def kernel(codebook, starts, ends, max_i):
    global last_exec_time_ns, last_result

    codebook = np.asarray(codebook)
    assert codebook.shape == (N, D), codebook.shape
    M = min(N, int(max_i) + 1)

    in_maps, sig, use_i2 = _prepare_inputs(codebook, starts, ends)

    key = (sig, use_i2)
    if key not in _programs:
        _programs[key] = _build_program(sig, use_i2)
    nc = _programs[key]

    results = _run_programs(nc, key, in_maps)

    pos_dev = np.empty(N, dtype=np.float32)
    neg_dev = np.empty(N, dtype=np.float32)
    for c in range(NCORES):
        s = results[c]["sums"]  # (T, 128, 2)
        off = c * ROWS_PER_CORE
        pos_dev[off:off + ROWS_PER_CORE] = s[..., 0].reshape(-1)
        neg_dev[off:off + ROWS_PER_CORE] = s[..., 1].reshape(-1)

    return np.asarray(_host_finalize(pos_dev, neg_dev, starts, ends, M))



# revision 4
# speedup vs baseline: 532.7359x; 532.7359x over previous
"""GroupAwareContrastiveLoss Trainium2 kernel (fp8 + fused-DVE version).

Strategy (sharding_hint: shard rows i across 8 cores, replicate codebook):
  - Host normalizes the codebook (zn = z/||z||), scales by SC=64 and
    quantizes to fp8 e4m3. Each core gets a column-rotated copy laid out
    [128, 8, N] so its own 1024 rows land in local columns [0, 1024) --
    the diagonal / range col-blocks are then identical across cores and
    the program stays SPMD while masks remain data-driven.
  - Device computes C = SC^2 * cos via fp8 DoubleRow matmuls (4 per
    128x512 tile, 256-deep contraction each), then ONE fused custom DVE
    op per tile: S = relu(|C| - SC^2*0.1)^2 with a fused row-sum
    accumulator (the full neg/ortho term, scaled by SC^4).
  - Band blocks (in-range cols + diagonal; host-detected signature) get:
    a masked-sum correction (custom TTR vs a host-built in_range|diag
    mask), and the pos chain d2 = sq_i + sq_j - 2*nrm_i*nrm_j*cos ->
    sqrt (ACT) -> fused relu^2*mask reduce (custom DVE).
  - Per-row sums return to host; host scales by 1/SC^4, adds the exact
    j==i ortho constant 0.81, does the O(M) counting/division/mean.
"""

import os
import sys
import numpy as np

if "/opt/trn_rl_repo" not in sys.path:
    sys.path.insert(0, "/opt/trn_rl_repo")

from contextlib import ExitStack
from operator import add as _op_add

import ml_dtypes

import concourse.bass as bass
import concourse.bacc as bacc
import concourse.mybir as mybir
from concourse import tile
from concourse.alu_op_type import AluOpType as ALU
from concourse.bass_utils import run_bass_kernel_spmd

N = 8192          # codebook rows (= cols of the cos matrix)
D = 1024          # feature dim
NCORES = 8
T = 8             # 128-row tiles per core
BLK = 512         # col-block width (one PSUM bank of fp32)
NBLK = N // BLK   # 16
KCH = D // 128    # 8 contraction chunks of 128
KG = KCH // 2     # 4 DoubleRow groups (256-deep each)
ROWS_PER_CORE = T * 128

M_POS = 0.5
M_NEG_SIM = 0.1
LAM_NEG = 1.0
SC = 64.0         # fp8 quantization scale for zn
SC2 = SC * SC
SC4 = SC2 * SC2

FP32 = mybir.dt.float32
BF16 = mybir.dt.bfloat16
FP8 = mybir.dt.float8e4
AF = mybir.ActivationFunctionType

_programs = {}
last_exec_time_ns = None
_last_run = None


# ---------------------------------------------------------------------------
# custom DVE ops (runtime-registered; same mechanism as dve_ops.OPS entries)
# ---------------------------------------------------------------------------
_custom_ops = None


def _get_custom_ops():
    global _custom_ops
    if _custom_ops is not None:
        return _custom_ops

    from concourse import dve_ops
    from concourse.dve_spec import (
        Spec, Src0, Src1, C2, Zero, lower, maxx, relu, sq,
    )
    from concourse.dve_uop import DveOpSpec

    def _sum_ref(body_fn):
        def _r(in0, in1, c0, c1, c2):
            b = body_fn(in0, in1, c0, c1, c2).astype(np.float32)
            return b, b.reshape(b.shape[0], -1).sum(axis=-1, keepdims=True)
        return _r

    def _dve_relu(x):
        return np.maximum(np.nan_to_num(x, nan=0.0, posinf=np.inf,
                                        neginf=-np.inf), 0)

    specs = [
        # out = relu(|x| - c2)^2 ; accum_out = row sum
        ("TENSOR_NEGABS_RELU_SQ_RED",
         Spec(
             body=sq(relu(maxx(Src0, Zero - Src0) - C2)),
             accum=_op_add,
             accum_init=Zero,
             reference=_sum_ref(
                 lambda in0, in1, c0, c1, c2:
                 _dve_relu(np.abs(in0.astype(np.float32)) - c2) ** 2),
         )),
        # out = relu(x - c2)^2 * mask ; accum_out = row sum
        ("TENSOR_POS_RELU_SQ_MASK_RED",
         Spec(
             body=sq(relu(Src0 - C2)) * Src1,
             accum=_op_add,
             accum_init=Zero,
             reference=_sum_ref(
                 lambda in0, in1, c0, c1, c2:
                 _dve_relu(in0.astype(np.float32) - c2) ** 2 * in1),
         )),
    ]

    made = []
    for name, spec in specs:
        existing = next((o for o in dve_ops.OPS if o.name == name), None)
        if existing is not None:
            made.append(existing)
            continue
        row = dve_ops._CUSTOM_DVE_ROW_BASE + len(dve_ops.OPS)
        assert row < 0x20, "custom-DVE opcode rows exhausted"
        dve_ops._SUB_OPCODE_FOR_NAME[name] = row
        shas = {}
        from concourse.dve_spec import _has_src1
        rd1 = _has_src1(spec)
        for ver in ("v3", "v4"):
            u = lower(spec, ver=ver)
            shas[ver] = DveOpSpec(name=name, opcode=row, uops=u,
                                  rd1_en=rd1).sha(ver)
        op = dve_ops.DveOp(name, spec, subdim=False, uops_sha=shas)
        dve_ops.OPS.append(op)
        dve_ops.CUSTOM_DVE_SPECS[name] = spec
        made.append(op)

    _custom_ops = tuple(made)
    return _custom_ops


# ---------------------------------------------------------------------------
# program builder
# ---------------------------------------------------------------------------
def _build_program(corr_sig, act_sig):
    """corr_sig/act_sig: tuple over t of sorted tuple of col-blocks that get
    the neg-correction / pos-chain ops (union across cores)."""
    NEG_OP, POS_OP = _get_custom_ops()

    nc = bacc.Bacc("TRN2", target_bir_lowering=False, debug=False,
                   num_devices=1)

    n_corr = sum(len(c) for c in corr_sig)
    n_act = sum(len(a) for a in act_sig)

    zn8 = nc.declare_dram_parameter("zn8", [128, KCH, N], FP8, isOutput=False)
    masks = nc.declare_dram_parameter(
        "masks", [max(n_corr, 1), 128, 2, BLK], BF16, isOutput=False)
    bc = nc.declare_dram_parameter(
        "bc", [max(n_act, 1), 128, 2, BLK], FP32, isOutput=False)
    scal = nc.declare_dram_parameter("scal", [T, 128, 2], FP32, isOutput=False)
    sums = nc.declare_dram_parameter("sums", [T, 128, 3], FP32, isOutput=True)

    dma = nc.sync.dma_start

    corr_idx = {}
    for t in range(T):
        for b in corr_sig[t]:
            corr_idx[(t, b)] = len(corr_idx)
    act_idx = {}
    for t in range(T):
        for b in act_sig[t]:
            act_idx[(t, b)] = len(act_idx)

    with tile.TileContext(nc) as tc, ExitStack() as ctx:
        res_pool = ctx.enter_context(tc.tile_pool(name="res", bufs=1))
        rhs_pool = ctx.enter_context(tc.tile_pool(name="rhs", bufs=2))
        psum_pool = ctx.enter_context(
            tc.tile_pool(name="psum", bufs=4, space="PSUM"))
        s_pool = ctx.enter_context(tc.tile_pool(name="spool", bufs=3))
        band_pool = ctx.enter_context(tc.tile_pool(name="band", bufs=2))

        # resident loads
        lhs = res_pool.tile([128, KCH, ROWS_PER_CORE], FP8, tag="lhs",
                            name="lhs")
        dma(lhs[:], zn8[:, :, 0:ROWS_PER_CORE])

        scal_sb = []
        for t in range(T):
            st = res_pool.tile([128, 2], FP32, tag=f"scal{t}", name=f"scal{t}")
            dma(st[:], scal[t])
            scal_sb.append(st)

        mask_sb = {}
        for (t, b), idx in corr_idx.items():
            mt = res_pool.tile([128, 2, BLK], BF16, tag=f"mk{idx}",
                               name=f"mk{idx}")
            dma(mt[:], masks[idx])
            mask_sb[(t, b)] = mt
        bc_sb = {}
        for (t, b), idx in act_idx.items():
            bt = res_pool.tile([128, 2, BLK], FP32, tag=f"bc{idx}",
                               name=f"bc{idx}")
            dma(bt[:], bc[idx])
            bc_sb[(t, b)] = bt

        negfull = [res_pool.tile([128, NBLK], FP32, tag=f"nf{t}", name=f"nf{t}")
                   for t in range(T)]
        negcorr = [res_pool.tile([128, max(len(corr_sig[t]), 1)], FP32,
                                 tag=f"ncr{t}", name=f"ncr{t}")
                   for t in range(T)]
        posacc = [res_pool.tile([128, max(len(act_sig[t]), 1)], FP32,
                                tag=f"pa{t}", name=f"pa{t}")
                  for t in range(T)]

    # main loop
        ncorr_col = [0] * T
        pos_col = [0] * T
        m_neg_dev = float(M_NEG_SIM * SC2)

        for b in range(NBLK):
            rhs = rhs_pool.tile([128, KCH, BLK], FP8, tag="rhs", name="rhs")
            dma(rhs[:], zn8[:, :, b * BLK:(b + 1) * BLK])

            for t in range(T):
                C = psum_pool.tile([128, BLK], FP32, tag="C", name="C")
                for g in range(KG):
                    nc.tensor.matmul(
                        C[:],
                        lhs[:, 2 * g:2 * g + 2, t * 128:(t + 1) * 128],
                        rhs[:, 2 * g:2 * g + 2, :],
                        start=(g == 0),
                        stop=(g == KG - 1),
                        perf_mode=mybir.MatmulPerfMode.DoubleRow,
                    )

                S = s_pool.tile([128, BLK], BF16, tag="S", name="S")
                nc.vector._custom_dve(
                    NEG_OP, out=S[:], in0=C[:], imm2=m_neg_dev,
                    accum_out=negfull[t][:, b:b + 1],
                )

                if (t, b) in corr_idx:
                    mt = mask_sb[(t, b)]
                    junk = s_pool.tile([128, BLK], BF16, tag="junk",
                                       name="junk")
                    from concourse.dve_ops import TENSOR_TENSOR_REDUCE
                    nc.vector._custom_dve(
                        TENSOR_TENSOR_REDUCE, out=junk[:], in0=S[:],
                        in1=mt[:, 0, :], s0=0.0, s1=1.0,
                        accum_out=negcorr[t][:, ncorr_col[t]:ncorr_col[t] + 1],
                    )
                    ncorr_col[t] += 1

                if (t, b) in act_idx:
                    bt = bc_sb[(t, b)]
                    st = scal_sb[t]
                    u = band_pool.tile([128, BLK], FP32, tag="u", name="u")
                    nc.vector.scalar_tensor_tensor(
                        u[:], in0=C[:], scalar=st[:, 0:1], in1=bt[:, 0, :],
                        op0=ALU.mult, op1=ALU.mult,
                    )
                    w = band_pool.tile([128, BLK], FP32, tag="w", name="w")
                    nc.vector.scalar_tensor_tensor(
                        w[:], in0=u[:], scalar=st[:, 1:2], in1=bt[:, 1, :],
                        op0=ALU.add, op1=ALU.add,
                    )
                    Dt = band_pool.tile([128, BLK], BF16, tag="Dt", name="Dt")
                    nc.scalar.activation(Dt[:], w[:], AF.Sqrt)
                    junk2 = s_pool.tile([128, BLK], BF16, tag="junk2",
                                        name="junk2")
                    mt = mask_sb[(t, b)]
                    nc.vector._custom_dve(
                        POS_OP, out=junk2[:], in0=Dt[:], in1=mt[:, 1, :],
                        imm2=float(M_POS),
                        accum_out=posacc[t][:, pos_col[t]:pos_col[t] + 1],
                    )
                    pos_col[t] += 1

        # finalize per row-tile: sums[t] = [pos, negfull_total, negcorr_total]
        for t in range(T):
            res = res_pool.tile([128, 3], FP32, tag=f"out{t}", name=f"out{t}")
            if pos_col[t] > 0:
                nc.vector.tensor_reduce(
                    res[:, 0:1], posacc[t][:, 0:pos_col[t]],
                    axis=mybir.AxisListType.X, op=ALU.add)
            else:
                nc.vector.memset(res[:, 0:1], 0.0)
            nc.vector.tensor_reduce(
                res[:, 1:2], negfull[t][:], axis=mybir.AxisListType.X,
                op=ALU.add)
            if ncorr_col[t] > 0:
                nc.vector.tensor_reduce(
                    res[:, 2:3], negcorr[t][:, 0:ncorr_col[t]],
                    axis=mybir.AxisListType.X, op=ALU.add)
            else:
                nc.vector.memset(res[:, 2:3], 0.0)
            dma(sums[t], res[:])

    nc.compile()
    return nc


# ---------------------------------------------------------------------------
# host-side input prep
# ---------------------------------------------------------------------------
def _prepare_inputs(codebook, starts, ends):
    cb = np.asarray(codebook, dtype=np.float32)
    s_arr = np.asarray(starts).astype(np.int64)
    e_arr = np.asarray(ends).astype(np.int64)

    sq64 = np.sum(cb.astype(np.float64) ** 2, axis=-1)
    nrm = np.sqrt(sq64).astype(np.float32)
    sq = sq64.astype(np.float32)
    zn = cb / nrm[:, None]
    zn8 = (zn * SC).astype(ml_dtypes.float8_e4m3)  # [N, D]

    s_cl = np.maximum(s_arr, 0)
    e_cl = np.minimum(e_arr, N - 1)
    nonempty = s_cl <= e_cl

    # ---- SPMD signature: union of needed blocks across cores ----
    corr_sig = [set() for _ in range(T)]
    act_sig = [set() for _ in range(T)]
    per_core = []
    for c in range(NCORES):
        off = c * ROWS_PER_CORE
        r = np.arange(ROWS_PER_CORE)
        gi = off + r
        sL = (s_cl[gi] - off) % N
        eL = (e_cl[gi] - off) % N
        wrap = nonempty[gi] & (sL > eL)
        ne = nonempty[gi]
        # interval list per row in local coords
        i1s = np.where(ne, np.where(wrap, 0, sL), 1)
        i1e = np.where(ne, eL, 0)
        i1v = ne.copy()
        i2s = np.where(wrap, sL, 1)
        i2e = np.where(wrap, np.int64(N - 1), 0)
        i2v = wrap.copy()
        per_core.append((off, i1s, i1e, i1v, i2s, i2e, i2v))
        for t in range(T):
            rt = slice(t * 128, (t + 1) * 128)
            for ss, ee, vv in ((i1s[rt], i1e[rt], i1v[rt]),
                               (i2s[rt], i2e[rt], i2v[rt])):
                ok = vv & (ss <= ee)
                if not ok.any():
                    continue
                for lo, hi in zip(ss[ok] // BLK, ee[ok] // BLK):
                    for bb in range(int(lo), int(hi) + 1):
                        act_sig[t].add(bb)
                        corr_sig[t].add(bb)
            corr_sig[t].add(t // 4)  # diagonal block always corrected

    corr_sig = tuple(tuple(sorted(s)) for s in corr_sig)
    act_sig = tuple(tuple(sorted(s)) for s in act_sig)

    corr_list = [(t, b) for t in range(T) for b in corr_sig[t]]
    act_list = [(t, b) for t in range(T) for b in act_sig[t]]
    n_corr, n_act = len(corr_list), len(act_list)

    # ---- per-core input maps ----
    in_maps = []
    for c in range(NCORES):
        off, i1s, i1e, i1v, i2s, i2e, i2v = per_core[c]
        # rotated fp8 matrix, layout [128, KCH, N]
        rolled = np.roll(zn8, -off, axis=0)              # [N, D]
        zn8_c = np.ascontiguousarray(
            rolled.T.reshape(KCH, 128, N).transpose(1, 0, 2))

        r = np.arange(ROWS_PER_CORE)
        gi = off + r

        scal_c = np.zeros((T, 128, 2), dtype=np.float32)
        flat = scal_c.reshape(ROWS_PER_CORE, 2)
        flat[:, 0] = -2.0 * nrm[gi] / SC2
        flat[:, 1] = sq[gi]

        # masks per corr entry: [n_corr, 128, 2, BLK] (mcorr, mpos)
        masks_c = np.zeros((max(n_corr, 1), 128, 2, BLK), dtype=np.float32)
        for idx, (t, b) in enumerate(corr_list):
            rt = np.arange(t * 128, (t + 1) * 128)
            cols = np.arange(b * BLK, (b + 1) * BLK)[None, :]
            inr = np.zeros((128, BLK), dtype=bool)
            for ss, ee, vv in ((i1s[rt], i1e[rt], i1v[rt]),
                               (i2s[rt], i2e[rt], i2v[rt])):
                inr |= vv[:, None] & (cols >= ss[:, None]) & \
                       (cols <= ee[:, None])
            diag = cols == rt[:, None]
            masks_c[idx, :, 0, :] = (inr | diag).astype(np.float32)
            masks_c[idx, :, 1, :] = (inr & ~diag).astype(np.float32)
        masks_c = masks_c.astype(ml_dtypes.bfloat16)

        # bc per act entry: [n_act, 128, 2, BLK] (nrm_j, sq_j broadcast)
        nrm_rot = np.roll(nrm, -off)
        sq_rot = np.roll(sq, -off)
        bc_c = np.zeros((max(n_act, 1), 128, 2, BLK), dtype=np.float32)
        for idx, (t, b) in enumerate(act_list):
            bc_c[idx, :, 0, :] = nrm_rot[b * BLK:(b + 1) * BLK][None, :]
            bc_c[idx, :, 1, :] = sq_rot[b * BLK:(b + 1) * BLK][None, :]

        in_maps.append({
            "zn8": zn8_c, "masks": masks_c, "bc": bc_c, "scal": scal_c,
        })

    return in_maps, corr_sig, act_sig


# ---------------------------------------------------------------------------
# host-side finalize
# ---------------------------------------------------------------------------
def _host_finalize(pos_dev, neg_dev, starts, ends, M):
    s_arr = np.asarray(starts).astype(np.int64)[:M]
    e_arr = np.asarray(ends).astype(np.int64)[:M]
    i_arr = np.arange(M, dtype=np.int64)

    lo = np.maximum(s_arr, 0)
    hi = np.minimum(e_arr, N - 1)
    cnt_in = np.maximum(0, hi - lo + 1)
    in_i = ((i_arr >= s_arr) & (i_arr <= e_arr)).astype(np.int64)
    pos_cnt = cnt_in - in_i
    neg_cnt = N - cnt_in + in_i

    diag_term = (1.0 - M_NEG_SIM) ** 2  # exact j==i ortho entry
    pos_sum = pos_dev[:M].astype(np.float64)
    neg_sum = neg_dev[:M].astype(np.float64) + diag_term

    pos_pull = pos_sum / np.maximum(pos_cnt, 1)
    ortho = neg_sum / np.maximum(neg_cnt, 1)
    valid = (pos_cnt > 0) & (neg_cnt > 0)
    per_row = np.where(valid, pos_pull + LAM_NEG * ortho, 0.0)
    cnt = int(valid.sum())
    if cnt > 0:
        return np.float32(per_row.sum() / cnt)
    return np.float32(0.0)


# ---------------------------------------------------------------------------
# NTFF trace hook (profiling only; inert when KTRACE is unset)
# ---------------------------------------------------------------------------
def _install_trace_hook():
    import types
    try:
        import antenv
        if "antenv.axon_hooks" not in sys.modules:
            mod = types.ModuleType("antenv.axon_hooks")
            state = {"hook": None}
            mod.set_axon_ntff_profile_hook = \
                lambda h: state.__setitem__("hook", h)
            mod.get_axon_ntff_profile_hook = lambda: state["hook"]
            sys.modules["antenv.axon_hooks"] = mod
            antenv.axon_hooks = mod
        from antenv.axon_hooks import (
            get_axon_ntff_profile_hook, set_axon_ntff_profile_hook,
        )
        if get_axon_ntff_profile_hook() is None:
            from trn_agent_boot.trn_boot import _ntff_profile_via_ctypes
            set_axon_ntff_profile_hook(
                _ntff_profile_via_ctypes("/opt/axon/libaxon_pjrt.so"))
        import concourse.bass_utils as bu
        if not getattr(bu.upload_artifacts, "_stubbed", False):
            def _noop_upload(tmpdir):
                return tmpdir
            _noop_upload._stubbed = True
            bu.upload_artifacts = _noop_upload
        return True
    except Exception:
        return False


# ---------------------------------------------------------------------------
# entry point
# ---------------------------------------------------------------------------
def kernel(codebook, starts, ends, max_i):
    global last_exec_time_ns, _last_run

    codebook = np.asarray(codebook)
    assert codebook.shape == (N, D), codebook.shape
    M = min(N, int(max_i) + 1)

    in_maps, corr_sig, act_sig = _prepare_inputs(codebook, starts, ends)

    key = (corr_sig, act_sig)
    if key not in _programs:
        _programs[key] = _build_program(corr_sig, act_sig)
    nc = _programs[key]

    trace = bool(os.environ.get("KTRACE"))
    if trace:
        trace = _install_trace_hook()
    res = run_bass_kernel_spmd(
        nc, in_maps, core_ids=list(range(NCORES)), trace=trace)
    last_exec_time_ns = res.exec_time_ns
    _last_run = res

    pos_dev = np.empty(N, dtype=np.float64)
    neg_dev = np.empty(N, dtype=np.float64)
    for c in range(NCORES):
        s = res.results[c]["sums"].astype(np.float64)  # (T, 128, 3)
        off = c * ROWS_PER_CORE
        pos_dev[off:off + ROWS_PER_CORE] = s[..., 0].reshape(-1)
        neg_dev[off:off + ROWS_PER_CORE] = \
            (s[..., 1] - s[..., 2]).reshape(-1) / SC4

    return np.asarray(_host_finalize(pos_dev, neg_dev, starts, ends, M))


# revision 11
# speedup vs baseline: 596.0840x; 1.1189x over previous
"""GroupAwareContrastiveLoss Trainium2 kernel (fp8 + fused-DVE version).

Strategy (sharding_hint: shard rows i across 8 cores, replicate codebook):
  - Host normalizes the codebook (zn = z/||z||), scales by SC=64 and
    quantizes to fp8 e4m3. Each core gets a column-rotated copy laid out
    [128, 8, N] so its own 1024 rows land in local columns [0, 1024) --
    the diagonal / range col-blocks are then identical across cores and
    the program stays SPMD while masks remain data-driven.
  - Device computes C = SC^2 * cos via fp8 DoubleRow matmuls (4 per
    128x512 tile, 256-deep contraction each), then ONE fused custom DVE
    op per tile: S = relu(|C| - SC^2*0.1)^2 with a fused row-sum
    accumulator (the full neg/ortho term, scaled by SC^4).
  - Band blocks (in-range cols + diagonal; host-detected signature) get:
    a masked-sum correction (custom TTR vs a host-built in_range|diag
    mask), and the pos chain d2 = sq_i + sq_j - 2*nrm_i*nrm_j*cos ->
    sqrt (ACT) -> fused relu^2*mask reduce (custom DVE).
  - Per-row sums return to host; host scales by 1/SC^4, adds the exact
    j==i ortho constant 0.81, does the O(M) counting/division/mean.
"""

import os
import sys
import numpy as np

if "/opt/trn_rl_repo" not in sys.path:
    sys.path.insert(0, "/opt/trn_rl_repo")

from contextlib import ExitStack
from operator import add as _op_add

import ml_dtypes

import concourse.bass as bass
import concourse.bacc as bacc
import concourse.mybir as mybir
from concourse import tile
from concourse.alu_op_type import AluOpType as ALU
from concourse.bass_utils import run_bass_kernel_spmd

N = 8192          # codebook rows (= cols of the cos matrix)
D = 1024          # feature dim
NCORES = 8
T = 8             # 128-row tiles per core
BLK = 512         # col-block width (one PSUM bank of fp32)
NBLK = N // BLK   # 16
KCH = D // 128    # 8 contraction chunks of 128
KG = KCH // 2     # 4 DoubleRow groups (256-deep each)
ROWS_PER_CORE = T * 128

M_POS = 0.5
M_NEG_SIM = 0.1
LAM_NEG = 1.0
SC = 64.0         # fp8 quantization scale for zn
SC2 = SC * SC
SC4 = SC2 * SC2

FP32 = mybir.dt.float32
BF16 = mybir.dt.bfloat16
FP8 = mybir.dt.float8e4
AF = mybir.ActivationFunctionType

_programs = {}
last_exec_time_ns = None
_last_run = None


# ---------------------------------------------------------------------------
# custom DVE ops (runtime-registered; same mechanism as dve_ops.OPS entries)
# ---------------------------------------------------------------------------
_custom_ops = None


def _get_custom_ops():
    global _custom_ops
    if _custom_ops is not None:
        return _custom_ops

    from concourse import dve_ops
    from concourse.dve_spec import (
        Spec, Src0, Src1, C0, C1, C2, Zero, lower, maxx, relu, sq,
    )
    from concourse.dve_uop import DveOpSpec

    def _sum_ref(body_fn):
        def _r(in0, in1, c0, c1, c2):
            b = body_fn(in0, in1, c0, c1, c2).astype(np.float32)
            return b, b.reshape(b.shape[0], -1).sum(axis=-1, keepdims=True)
        return _r

    def _dve_relu(x):
        return np.maximum(np.nan_to_num(x, nan=0.0, posinf=np.inf,
                                        neginf=-np.inf), 0)

    specs = [
        # out = relu(|x| - c2)^2 ; accum_out = row sum
        ("TENSOR_NEGABS_RELU_SQ_RED",
         Spec(
             body=sq(relu(maxx(Src0, Zero - Src0) - C2)),
             accum=_op_add,
             accum_init=Zero,
             reference=_sum_ref(
                 lambda in0, in1, c0, c1, c2:
                 _dve_relu(np.abs(in0.astype(np.float32)) - c2) ** 2),
         )),
        # out = relu(x - c2)^2 * mask ; accum_out = row sum
        ("TENSOR_POS_RELU_SQ_MASK_RED",
         Spec(
             body=sq(relu(Src0 - C2)) * Src1,
             accum=_op_add,
             accum_init=Zero,
             reference=_sum_ref(
                 lambda in0, in1, c0, c1, c2:
                 _dve_relu(in0.astype(np.float32) - c2) ** 2 * in1),
         )),
        # out = (x*c0)*y + y^2 + c1   (d2 from scaled cos + nrm_j in one op)
        ("TENSOR_D2_FROM_COS",
         Spec(
             body=(Src0 * C0) * Src1 + sq(Src1) + C1,
             reference=lambda in0, in1, c0, c1, c2: (
                 (in0.astype(np.float32) * c0) * in1 + in1 * in1 + c1
             ).astype(np.float32),
         )),
    ]

    made = []
    for name, spec in specs:
        existing = next((o for o in dve_ops.OPS if o.name == name), None)
        if existing is not None:
            made.append(existing)
            continue
        row = dve_ops._CUSTOM_DVE_ROW_BASE + len(dve_ops.OPS)
        assert row < 0x20, "custom-DVE opcode rows exhausted"
        dve_ops._SUB_OPCODE_FOR_NAME[name] = row
        shas = {}
        from concourse.dve_spec import _has_src1
        rd1 = _has_src1(spec)
        for ver in ("v3", "v4"):
            u = lower(spec, ver=ver)
            shas[ver] = DveOpSpec(name=name, opcode=row, uops=u,
                                  rd1_en=rd1).sha(ver)
        op = dve_ops.DveOp(name, spec, subdim=False, uops_sha=shas)
        dve_ops.OPS.append(op)
        dve_ops.CUSTOM_DVE_SPECS[name] = spec
        made.append(op)

    _custom_ops = tuple(made)
    return _custom_ops


# ---------------------------------------------------------------------------
# program builder
# ---------------------------------------------------------------------------
def _build_program(corr_sig, act_sig):
    """corr_sig/act_sig: tuple over t of sorted tuple of col-blocks that get
    the neg-correction / pos-chain ops (union across cores)."""
    NEG_OP, POS_OP, D2_OP = _get_custom_ops()

    nc = bacc.Bacc("TRN2", target_bir_lowering=False, debug=False,
                   num_devices=1)

    n_corr = sum(len(c) for c in corr_sig)
    n_act = sum(len(a) for a in act_sig)

    zn8 = nc.declare_dram_parameter("zn8", [128, KCH, N], FP8, isOutput=False)
    masks = nc.declare_dram_parameter(
        "masks", [max(n_corr, 1), 128, 2, BLK], BF16, isOutput=False)
    bc = nc.declare_dram_parameter(
        "bc", [max(n_act, 1), 128, BLK], FP32, isOutput=False)
    scal = nc.declare_dram_parameter("scal", [T, 128, 2], FP32, isOutput=False)
    sums = nc.declare_dram_parameter("sums", [T, 128, 3], FP32, isOutput=True)

    dma = nc.sync.dma_start      # rhs stream + outputs
    dma_aux = nc.gpsimd.dma_start  # resident loads, off the critical queue

    corr_idx = {}
    for t in range(T):
        for b in corr_sig[t]:
            corr_idx[(t, b)] = len(corr_idx)
    act_idx = {}
    for t in range(T):
        for b in act_sig[t]:
            act_idx[(t, b)] = len(act_idx)

    with tile.TileContext(nc) as tc, ExitStack() as ctx:
        res_pool = ctx.enter_context(tc.tile_pool(name="res", bufs=1))
        rhs_pool = ctx.enter_context(tc.tile_pool(name="rhs", bufs=2))
        psum_pool = ctx.enter_context(
            tc.tile_pool(name="psum", bufs=4, space="PSUM"))
        s_pool = ctx.enter_context(tc.tile_pool(name="spool", bufs=3))
        band_pool = ctx.enter_context(tc.tile_pool(name="band", bufs=2))

        # resident loads (aux DMA queue so the rhs stream starts immediately)
        lhs = res_pool.tile([128, KCH, ROWS_PER_CORE], FP8, tag="lhs",
                            name="lhs")
        dma_aux(lhs[:], zn8[:, :, 0:ROWS_PER_CORE])

        scal_sb = []
        for t in range(T):
            st = res_pool.tile([128, 2], FP32, tag=f"scal{t}", name=f"scal{t}")
            dma_aux(st[:], scal[t])
            scal_sb.append(st)

        mask_sb = {}
        for (t, b), idx in corr_idx.items():
            mt = res_pool.tile([128, 2, BLK], BF16, tag=f"mk{idx}",
                               name=f"mk{idx}")
            dma_aux(mt[:], masks[idx])
            mask_sb[(t, b)] = mt
        bc_sb = {}
        for (t, b), idx in act_idx.items():
            bt = res_pool.tile([128, BLK], FP32, tag=f"bc{idx}",
                               name=f"bc{idx}")
            dma_aux(bt[:], bc[idx])
            bc_sb[(t, b)] = bt

        negfull = [res_pool.tile([128, NBLK], FP32, tag=f"nf{t}", name=f"nf{t}")
                   for t in range(T)]
        negcorr = [res_pool.tile([128, max(len(corr_sig[t]), 1)], FP32,
                                 tag=f"ncr{t}", name=f"ncr{t}")
                   for t in range(T)]
        posacc = [res_pool.tile([128, max(len(act_sig[t]), 1)], FP32,
                                tag=f"pa{t}", name=f"pa{t}")
                  for t in range(T)]

    # main loop
        ncorr_col = [0] * T
        pos_col = [0] * T
        m_neg_dev = float(M_NEG_SIM * SC2)

        for b in range(NBLK):
            rhs = rhs_pool.tile([128, KCH, BLK], FP8, tag="rhs", name="rhs")
            dma(rhs[:], zn8[:, :, b * BLK:(b + 1) * BLK])

            for t in range(T):
                C = psum_pool.tile([128, BLK], FP32, tag="C", name="C")
                for g in range(KG):
                    nc.tensor.matmul(
                        C[:],
                        lhs[:, 2 * g:2 * g + 2, t * 128:(t + 1) * 128],
                        rhs[:, 2 * g:2 * g + 2, :],
                        start=(g == 0),
                        stop=(g == KG - 1),
                        perf_mode=mybir.MatmulPerfMode.DoubleRow,
                    )

                S = s_pool.tile([128, BLK], BF16, tag="S", name="S")
                nc.vector._custom_dve(
                    NEG_OP, out=S[:], in0=C[:], imm2=m_neg_dev,
                    accum_out=negfull[t][:, b:b + 1],
                )

                if (t, b) in corr_idx:
                    mt = mask_sb[(t, b)]
                    junk = s_pool.tile([128, BLK], BF16, tag="junk",
                                       name="junk")
                    from concourse.dve_ops import TENSOR_TENSOR_REDUCE
                    nc.vector._custom_dve(
                        TENSOR_TENSOR_REDUCE, out=junk[:], in0=S[:],
                        in1=mt[:, 0, :], s0=0.0, s1=1.0,
                        accum_out=negcorr[t][:, ncorr_col[t]:ncorr_col[t] + 1],
                    )
                    ncorr_col[t] += 1

                if (t, b) in act_idx:
                    bt = bc_sb[(t, b)]
                    st = scal_sb[t]
                    w = band_pool.tile([128, BLK], FP32, tag="w", name="w")
                    nc.vector._custom_dve(
                        D2_OP, out=w[:], in0=C[:], in1=bt[:],
                        s0=st[:, 0:1], s1=st[:, 1:2],
                    )
                    Dt = band_pool.tile([128, BLK], BF16, tag="Dt", name="Dt")
                    nc.scalar.activation(Dt[:], w[:], AF.Sqrt)
                    junk2 = s_pool.tile([128, BLK], BF16, tag="junk2",
                                        name="junk2")
                    mt = mask_sb[(t, b)]
                    nc.vector._custom_dve(
                        POS_OP, out=junk2[:], in0=Dt[:], in1=mt[:, 1, :],
                        imm2=float(M_POS),
                        accum_out=posacc[t][:, pos_col[t]:pos_col[t] + 1],
                    )
                    pos_col[t] += 1

        # finalize per row-tile: sums[t] = [pos, negfull_total, negcorr_total]
        for t in range(T):
            res = res_pool.tile([128, 3], FP32, tag=f"out{t}", name=f"out{t}")
            if pos_col[t] > 0:
                nc.vector.tensor_reduce(
                    res[:, 0:1], posacc[t][:, 0:pos_col[t]],
                    axis=mybir.AxisListType.X, op=ALU.add)
            else:
                nc.vector.memset(res[:, 0:1], 0.0)
            nc.vector.tensor_reduce(
                res[:, 1:2], negfull[t][:], axis=mybir.AxisListType.X,
                op=ALU.add)
            if ncorr_col[t] > 0:
                nc.vector.tensor_reduce(
                    res[:, 2:3], negcorr[t][:, 0:ncorr_col[t]],
                    axis=mybir.AxisListType.X, op=ALU.add)
            else:
                nc.vector.memset(res[:, 2:3], 0.0)
            dma(sums[t], res[:])

    nc.compile()
    return nc


# ---------------------------------------------------------------------------
# host-side input prep
# ---------------------------------------------------------------------------
def _prepare_inputs(codebook, starts, ends):
    cb = np.asarray(codebook, dtype=np.float32)
    s_arr = np.asarray(starts).astype(np.int64)
    e_arr = np.asarray(ends).astype(np.int64)

    sq64 = np.sum(cb.astype(np.float64) ** 2, axis=-1)
    nrm = np.sqrt(sq64).astype(np.float32)
    sq = sq64.astype(np.float32)
    zn = cb / nrm[:, None]
    zn8 = (zn * SC).astype(ml_dtypes.float8_e4m3)  # [N, D]

    s_cl = np.maximum(s_arr, 0)
    e_cl = np.minimum(e_arr, N - 1)
    nonempty = s_cl <= e_cl

    # ---- SPMD signature: union of needed blocks across cores ----
    corr_sig = [set() for _ in range(T)]
    act_sig = [set() for _ in range(T)]
    per_core = []
    for c in range(NCORES):
        off = c * ROWS_PER_CORE
        r = np.arange(ROWS_PER_CORE)
        gi = off + r
        sL = (s_cl[gi] - off) % N
        eL = (e_cl[gi] - off) % N
        wrap = nonempty[gi] & (sL > eL)
        ne = nonempty[gi]
        # interval list per row in local coords
        i1s = np.where(ne, np.where(wrap, 0, sL), 1)
        i1e = np.where(ne, eL, 0)
        i1v = ne.copy()
        i2s = np.where(wrap, sL, 1)
        i2e = np.where(wrap, np.int64(N - 1), 0)
        i2v = wrap.copy()
        per_core.append((off, i1s, i1e, i1v, i2s, i2e, i2v))
        for t in range(T):
            rt = slice(t * 128, (t + 1) * 128)
            for ss, ee, vv in ((i1s[rt], i1e[rt], i1v[rt]),
                               (i2s[rt], i2e[rt], i2v[rt])):
                ok = vv & (ss <= ee)
                if not ok.any():
                    continue
                for lo, hi in zip(ss[ok] // BLK, ee[ok] // BLK):
                    for bb in range(int(lo), int(hi) + 1):
                        act_sig[t].add(bb)
                        corr_sig[t].add(bb)
            corr_sig[t].add(t // 4)  # diagonal block always corrected

    corr_sig = tuple(tuple(sorted(s)) for s in corr_sig)
    act_sig = tuple(tuple(sorted(s)) for s in act_sig)

    corr_list = [(t, b) for t in range(T) for b in corr_sig[t]]
    act_list = [(t, b) for t in range(T) for b in act_sig[t]]
    n_corr, n_act = len(corr_list), len(act_list)

    # ---- per-core input maps ----
    in_maps = []
    for c in range(NCORES):
        off, i1s, i1e, i1v, i2s, i2e, i2v = per_core[c]
        # rotated fp8 matrix, layout [128, KCH, N]
        rolled = np.roll(zn8, -off, axis=0)              # [N, D]
        zn8_c = np.ascontiguousarray(
            rolled.T.reshape(KCH, 128, N).transpose(1, 0, 2))

        r = np.arange(ROWS_PER_CORE)
        gi = off + r

        scal_c = np.zeros((T, 128, 2), dtype=np.float32)
        flat = scal_c.reshape(ROWS_PER_CORE, 2)
        flat[:, 0] = -2.0 * nrm[gi] / SC2
        flat[:, 1] = sq[gi]

        # masks per corr entry: [n_corr, 128, 2, BLK] (mcorr, mpos)
        masks_c = np.zeros((max(n_corr, 1), 128, 2, BLK), dtype=np.float32)
        for idx, (t, b) in enumerate(corr_list):
            rt = np.arange(t * 128, (t + 1) * 128)
            cols = np.arange(b * BLK, (b + 1) * BLK)[None, :]
            inr = np.zeros((128, BLK), dtype=bool)
            for ss, ee, vv in ((i1s[rt], i1e[rt], i1v[rt]),
                               (i2s[rt], i2e[rt], i2v[rt])):
                inr |= vv[:, None] & (cols >= ss[:, None]) & \
                       (cols <= ee[:, None])
            diag = cols == rt[:, None]
            masks_c[idx, :, 0, :] = (inr | diag).astype(np.float32)
            masks_c[idx, :, 1, :] = (inr & ~diag).astype(np.float32)
        masks_c = masks_c.astype(ml_dtypes.bfloat16)

        # bc per act entry: [n_act, 128, BLK] (nrm_j broadcast; sq_j = nrm^2
        # is recomputed on-device inside the fused d2 op)
        nrm_rot = np.roll(nrm, -off)
        bc_c = np.zeros((max(n_act, 1), 128, BLK), dtype=np.float32)
        for idx, (t, b) in enumerate(act_list):
            bc_c[idx] = nrm_rot[b * BLK:(b + 1) * BLK][None, :]

        in_maps.append({
            "zn8": zn8_c, "masks": masks_c, "bc": bc_c, "scal": scal_c,
        })

    return in_maps, corr_sig, act_sig


# ---------------------------------------------------------------------------
# host-side finalize
# ---------------------------------------------------------------------------
def _host_finalize(pos_dev, neg_dev, starts, ends, M):
    s_arr = np.asarray(starts).astype(np.int64)[:M]
    e_arr = np.asarray(ends).astype(np.int64)[:M]
    i_arr = np.arange(M, dtype=np.int64)

    lo = np.maximum(s_arr, 0)
    hi = np.minimum(e_arr, N - 1)
    cnt_in = np.maximum(0, hi - lo + 1)
    in_i = ((i_arr >= s_arr) & (i_arr <= e_arr)).astype(np.int64)
    pos_cnt = cnt_in - in_i
    neg_cnt = N - cnt_in + in_i

    diag_term = (1.0 - M_NEG_SIM) ** 2  # exact j==i ortho entry
    pos_sum = pos_dev[:M].astype(np.float64)
    neg_sum = neg_dev[:M].astype(np.float64) + diag_term

    pos_pull = pos_sum / np.maximum(pos_cnt, 1)
    ortho = neg_sum / np.maximum(neg_cnt, 1)
    valid = (pos_cnt > 0) & (neg_cnt > 0)
    per_row = np.where(valid, pos_pull + LAM_NEG * ortho, 0.0)
    cnt = int(valid.sum())
    if cnt > 0:
        return np.float32(per_row.sum() / cnt)
    return np.float32(0.0)


# ---------------------------------------------------------------------------
# NTFF trace hook (profiling only; inert when KTRACE is unset)
# ---------------------------------------------------------------------------
def _install_trace_hook():
    import types
    try:
        import antenv
        if "antenv.axon_hooks" not in sys.modules:
            mod = types.ModuleType("antenv.axon_hooks")
            state = {"hook": None}
            mod.set_axon_ntff_profile_hook = \
                lambda h: state.__setitem__("hook", h)
            mod.get_axon_ntff_profile_hook = lambda: state["hook"]
            sys.modules["antenv.axon_hooks"] = mod
            antenv.axon_hooks = mod
        from antenv.axon_hooks import (
            get_axon_ntff_profile_hook, set_axon_ntff_profile_hook,
        )
        if get_axon_ntff_profile_hook() is None:
            from trn_agent_boot.trn_boot import _ntff_profile_via_ctypes
            set_axon_ntff_profile_hook(
                _ntff_profile_via_ctypes("/opt/axon/libaxon_pjrt.so"))
        import concourse.bass_utils as bu
        if not getattr(bu.upload_artifacts, "_stubbed", False):
            def _noop_upload(tmpdir):
                return tmpdir
            _noop_upload._stubbed = True
            bu.upload_artifacts = _noop_upload
        return True
    except Exception:
        return False


# ---------------------------------------------------------------------------
# entry point
# ---------------------------------------------------------------------------
def kernel(codebook, starts, ends, max_i):
    global last_exec_time_ns, _last_run

    codebook = np.asarray(codebook)
    assert codebook.shape == (N, D), codebook.shape
    M = min(N, int(max_i) + 1)

    in_maps, corr_sig, act_sig = _prepare_inputs(codebook, starts, ends)

    key = (corr_sig, act_sig)
    if key not in _programs:
        _programs[key] = _build_program(corr_sig, act_sig)
    nc = _programs[key]

    trace = bool(os.environ.get("KTRACE"))
    if trace:
        trace = _install_trace_hook()
    res = run_bass_kernel_spmd(
        nc, in_maps, core_ids=list(range(NCORES)), trace=trace)
    last_exec_time_ns = res.exec_time_ns
    _last_run = res

    pos_dev = np.empty(N, dtype=np.float64)
    neg_dev = np.empty(N, dtype=np.float64)
    for c in range(NCORES):
        s = res.results[c]["sums"].astype(np.float64)  # (T, 128, 3)
        off = c * ROWS_PER_CORE
        pos_dev[off:off + ROWS_PER_CORE] = s[..., 0].reshape(-1)
        neg_dev[off:off + ROWS_PER_CORE] = \
            (s[..., 1] - s[..., 2]).reshape(-1) / SC4

    return np.asarray(_host_finalize(pos_dev, neg_dev, starts, ends, M))


# revision 13
# speedup vs baseline: 631.4096x; 1.0593x over previous
"""GroupAwareContrastiveLoss Trainium2 kernel (fp8 + fused-DVE version).

Strategy (sharding_hint: shard rows i across 8 cores, replicate codebook):
  - Host normalizes the codebook (zn = z/||z||), scales by SC=64 and
    quantizes to fp8 e4m3. Each core gets a column-rotated copy laid out
    [128, 8, N] so its own 1024 rows land in local columns [0, 1024) --
    the diagonal / range col-blocks are then identical across cores and
    the program stays SPMD while masks remain data-driven.
  - Device computes C = SC^2 * cos via fp8 DoubleRow matmuls (4 per
    128x512 tile, 256-deep contraction each), then ONE fused custom DVE
    op per tile: S = relu(|C| - SC^2*0.1)^2 with a fused row-sum
    accumulator (the full neg/ortho term, scaled by SC^4).
  - Band blocks (in-range cols + diagonal; host-detected signature) get:
    a masked-sum correction (custom TTR vs a host-built in_range|diag
    mask), and the pos chain d2 = sq_i + sq_j - 2*nrm_i*nrm_j*cos ->
    sqrt (ACT) -> fused relu^2*mask reduce (custom DVE).
  - Per-row sums return to host; host scales by 1/SC^4, adds the exact
    j==i ortho constant 0.81, does the O(M) counting/division/mean.
"""

import os
import sys
import numpy as np

if "/opt/trn_rl_repo" not in sys.path:
    sys.path.insert(0, "/opt/trn_rl_repo")

from contextlib import ExitStack
from operator import add as _op_add

import ml_dtypes

import concourse.bass as bass
import concourse.bacc as bacc
import concourse.mybir as mybir
from concourse import tile
from concourse.alu_op_type import AluOpType as ALU
from concourse.bass_utils import run_bass_kernel_spmd

N = 8192          # codebook rows (= cols of the cos matrix)
D = 1024          # feature dim
NCORES = 8
T = 8             # 128-row tiles per core
BLK = 512         # col-block width (one PSUM bank of fp32)
NBLK = N // BLK   # 16
KCH = D // 128    # 8 contraction chunks of 128
KG = KCH // 2     # 4 DoubleRow groups (256-deep each)
ROWS_PER_CORE = T * 128

M_POS = 0.5
M_NEG_SIM = 0.1
LAM_NEG = 1.0
SC = 64.0         # fp8 quantization scale for zn
SC2 = SC * SC
SC4 = SC2 * SC2

FP32 = mybir.dt.float32
BF16 = mybir.dt.bfloat16
FP8 = mybir.dt.float8e4
AF = mybir.ActivationFunctionType

_programs = {}
last_exec_time_ns = None
_last_run = None


# ---------------------------------------------------------------------------
# custom DVE ops (runtime-registered; same mechanism as dve_ops.OPS entries)
# ---------------------------------------------------------------------------
_custom_ops = None


def _get_custom_ops():
    global _custom_ops
    if _custom_ops is not None:
        return _custom_ops

    from concourse import dve_ops
    from concourse.dve_spec import (
        Spec, Src0, Src1, C0, C1, C2, Zero, lower, maxx, relu, sq,
    )
    from concourse.dve_uop import DveOpSpec

    def _sum_ref(body_fn):
        def _r(in0, in1, c0, c1, c2):
            b = body_fn(in0, in1, c0, c1, c2).astype(np.float32)
            return b, b.reshape(b.shape[0], -1).sum(axis=-1, keepdims=True)
        return _r

    def _dve_relu(x):
        return np.maximum(np.nan_to_num(x, nan=0.0, posinf=np.inf,
                                        neginf=-np.inf), 0)

    specs = [
        # out = relu(|x| - c2)^2 ; accum_out = row sum
        ("TENSOR_NEGABS_RELU_SQ_RED",
         Spec(
             body=sq(relu(maxx(Src0, Zero - Src0) - C2)),
             accum=_op_add,
             accum_init=Zero,
             reference=_sum_ref(
                 lambda in0, in1, c0, c1, c2:
                 _dve_relu(np.abs(in0.astype(np.float32)) - c2) ** 2),
         )),
        # out = relu(x - c2)^2 * mask ; accum_out = row sum
        ("TENSOR_POS_RELU_SQ_MASK_RED",
         Spec(
             body=sq(relu(Src0 - C2)) * Src1,
             accum=_op_add,
             accum_init=Zero,
             reference=_sum_ref(
                 lambda in0, in1, c0, c1, c2:
                 _dve_relu(in0.astype(np.float32) - c2) ** 2 * in1),
         )),
        # out = (x*c0)*y + y^2 + c1   (d2 from scaled cos + nrm_j in one op)
        ("TENSOR_D2_FROM_COS",
         Spec(
             body=(Src0 * C0) * Src1 + sq(Src1) + C1,
             reference=lambda in0, in1, c0, c1, c2: (
                 (in0.astype(np.float32) * c0) * in1 + in1 * in1 + c1
             ).astype(np.float32),
         )),
    ]

    made = []
    for name, spec in specs:
        existing = next((o for o in dve_ops.OPS if o.name == name), None)
        if existing is not None:
            made.append(existing)
            continue
        row = dve_ops._CUSTOM_DVE_ROW_BASE + len(dve_ops.OPS)
        assert row < 0x20, "custom-DVE opcode rows exhausted"
        dve_ops._SUB_OPCODE_FOR_NAME[name] = row
        shas = {}
        from concourse.dve_spec import _has_src1
        rd1 = _has_src1(spec)
        for ver in ("v3", "v4"):
            u = lower(spec, ver=ver)
            shas[ver] = DveOpSpec(name=name, opcode=row, uops=u,
                                  rd1_en=rd1).sha(ver)
        op = dve_ops.DveOp(name, spec, subdim=False, uops_sha=shas)
        dve_ops.OPS.append(op)
        dve_ops.CUSTOM_DVE_SPECS[name] = spec
        made.append(op)

    _custom_ops = tuple(made)
    return _custom_ops


# ---------------------------------------------------------------------------
# program builder
# ---------------------------------------------------------------------------
def _build_program(corr_sig, act_sig):
    """corr_sig/act_sig: tuple over t of sorted tuple of col-blocks that get
    the neg-correction / pos-chain ops (union across cores)."""
    NEG_OP, POS_OP, D2_OP = _get_custom_ops()

    nc = bacc.Bacc("TRN2", target_bir_lowering=False, debug=False,
                   num_devices=1)

    n_corr = sum(len(c) for c in corr_sig)
    n_act = sum(len(a) for a in act_sig)

    zn8 = nc.declare_dram_parameter("zn8", [128, KCH, N], FP8, isOutput=False)
    masks = nc.declare_dram_parameter(
        "masks", [max(n_corr, 1), 128, 2, BLK], BF16, isOutput=False)
    bc = nc.declare_dram_parameter(
        "bc", [max(n_act, 1), 128, BLK], FP32, isOutput=False)
    scal = nc.declare_dram_parameter("scal", [T, 128, 2], FP32, isOutput=False)
    sums = nc.declare_dram_parameter("sums", [T, 128, 3], FP32, isOutput=True)

    dma = nc.sync.dma_start      # rhs stream + outputs
    dma_aux = nc.gpsimd.dma_start  # resident loads, off the critical queue

    corr_idx = {}
    for t in range(T):
        for b in corr_sig[t]:
            corr_idx[(t, b)] = len(corr_idx)
    act_idx = {}
    for t in range(T):
        for b in act_sig[t]:
            act_idx[(t, b)] = len(act_idx)

    with tile.TileContext(nc) as tc, ExitStack() as ctx:
        res_pool = ctx.enter_context(tc.tile_pool(name="res", bufs=1))
        rhs_pool = ctx.enter_context(tc.tile_pool(name="rhs", bufs=2))
        psum_pool = ctx.enter_context(
            tc.tile_pool(name="psum", bufs=8, space="PSUM"))
        s_pool = ctx.enter_context(tc.tile_pool(name="spool", bufs=3))
        band_pool = ctx.enter_context(tc.tile_pool(name="band", bufs=2))

        # lhs on the fast sync queue (first matmul blocks on it); other
        # resident loads go to the gpsimd queue so the rhs stream starts
        # immediately after lhs.
        lhs = res_pool.tile([128, KCH, ROWS_PER_CORE], FP8, tag="lhs",
                            name="lhs")
        dma(lhs[:], zn8[:, :, 0:ROWS_PER_CORE])

        scal_sb = []
        for t in range(T):
            st = res_pool.tile([128, 2], FP32, tag=f"scal{t}", name=f"scal{t}")
            dma_aux(st[:], scal[t])
            scal_sb.append(st)

        mask_sb = {}
        for (t, b), idx in corr_idx.items():
            mt = res_pool.tile([128, 2, BLK], BF16, tag=f"mk{idx}",
                               name=f"mk{idx}")
            dma_aux(mt[:], masks[idx])
            mask_sb[(t, b)] = mt
        bc_sb = {}
        for (t, b), idx in act_idx.items():
            bt = res_pool.tile([128, BLK], FP32, tag=f"bc{idx}",
                               name=f"bc{idx}")
            dma_aux(bt[:], bc[idx])
            bc_sb[(t, b)] = bt

        negfull = [res_pool.tile([128, NBLK], FP32, tag=f"nf{t}", name=f"nf{t}")
                   for t in range(T)]
        negcorr = [res_pool.tile([128, max(len(corr_sig[t]), 1)], FP32,
                                 tag=f"ncr{t}", name=f"ncr{t}")
                   for t in range(T)]
        posacc = [res_pool.tile([128, max(len(act_sig[t]), 1)], FP32,
                                tag=f"pa{t}", name=f"pa{t}")
                  for t in range(T)]

    # main loop
        ncorr_col = [0] * T
        pos_col = [0] * T
        m_neg_dev = float(M_NEG_SIM * SC2)

        for b in range(NBLK):
            rhs = rhs_pool.tile([128, KCH, BLK], FP8, tag="rhs", name="rhs")
            dma(rhs[:], zn8[:, :, b * BLK:(b + 1) * BLK])

            for t in range(T):
                C = psum_pool.tile([128, BLK], FP32, tag="C", name="C")
                for g in range(KG):
                    nc.tensor.matmul(
                        C[:],
                        lhs[:, 2 * g:2 * g + 2, t * 128:(t + 1) * 128],
                        rhs[:, 2 * g:2 * g + 2, :],
                        start=(g == 0),
                        stop=(g == KG - 1),
                        perf_mode=mybir.MatmulPerfMode.DoubleRow,
                    )

                S = s_pool.tile([128, BLK], BF16, tag="S", name="S")
                nc.vector._custom_dve(
                    NEG_OP, out=S[:], in0=C[:], imm2=m_neg_dev,
                    accum_out=negfull[t][:, b:b + 1],
                )

                if (t, b) in corr_idx:
                    mt = mask_sb[(t, b)]
                    junk = s_pool.tile([128, BLK], BF16, tag="junk",
                                       name="junk")
                    from concourse.dve_ops import TENSOR_TENSOR_REDUCE
                    nc.vector._custom_dve(
                        TENSOR_TENSOR_REDUCE, out=junk[:], in0=S[:],
                        in1=mt[:, 0, :], s0=0.0, s1=1.0,
                        accum_out=negcorr[t][:, ncorr_col[t]:ncorr_col[t] + 1],
                    )
                    ncorr_col[t] += 1

                if (t, b) in act_idx:
                    bt = bc_sb[(t, b)]
                    st = scal_sb[t]
                    w = band_pool.tile([128, BLK], FP32, tag="w", name="w")
                    nc.vector._custom_dve(
                        D2_OP, out=w[:], in0=C[:], in1=bt[:],
                        s0=st[:, 0:1], s1=st[:, 1:2],
                    )
                    Dt = band_pool.tile([128, BLK], BF16, tag="Dt", name="Dt")
                    nc.scalar.activation(Dt[:], w[:], AF.Sqrt)
                    junk2 = s_pool.tile([128, BLK], BF16, tag="junk2",
                                        name="junk2")
                    mt = mask_sb[(t, b)]
                    nc.vector._custom_dve(
                        POS_OP, out=junk2[:], in0=Dt[:], in1=mt[:, 1, :],
                        imm2=float(M_POS),
                        accum_out=posacc[t][:, pos_col[t]:pos_col[t] + 1],
                    )
                    pos_col[t] += 1

        # finalize per row-tile: sums[t] = [pos, negfull_total, negcorr_total]
        for t in range(T):
            res = res_pool.tile([128, 3], FP32, tag=f"out{t}", name=f"out{t}")
            if pos_col[t] > 0:
                nc.vector.tensor_reduce(
                    res[:, 0:1], posacc[t][:, 0:pos_col[t]],
                    axis=mybir.AxisListType.X, op=ALU.add)
            else:
                nc.vector.memset(res[:, 0:1], 0.0)
            nc.vector.tensor_reduce(
                res[:, 1:2], negfull[t][:], axis=mybir.AxisListType.X,
                op=ALU.add)
            if ncorr_col[t] > 0:
                nc.vector.tensor_reduce(
                    res[:, 2:3], negcorr[t][:, 0:ncorr_col[t]],
                    axis=mybir.AxisListType.X, op=ALU.add)
            else:
                nc.vector.memset(res[:, 2:3], 0.0)
            dma(sums[t], res[:])

    nc.compile()
    return nc


# ---------------------------------------------------------------------------
# host-side input prep
# ---------------------------------------------------------------------------
def _prepare_inputs(codebook, starts, ends):
    cb = np.asarray(codebook, dtype=np.float32)
    s_arr = np.asarray(starts).astype(np.int64)
    e_arr = np.asarray(ends).astype(np.int64)

    sq64 = np.sum(cb.astype(np.float64) ** 2, axis=-1)
    nrm = np.sqrt(sq64).astype(np.float32)
    sq = sq64.astype(np.float32)
    zn = cb / nrm[:, None]
    zn8 = (zn * SC).astype(ml_dtypes.float8_e4m3)  # [N, D]

    s_cl = np.maximum(s_arr, 0)
    e_cl = np.minimum(e_arr, N - 1)
    nonempty = s_cl <= e_cl

    # ---- SPMD signature: union of needed blocks across cores ----
    corr_sig = [set() for _ in range(T)]
    act_sig = [set() for _ in range(T)]
    per_core = []
    for c in range(NCORES):
        off = c * ROWS_PER_CORE
        r = np.arange(ROWS_PER_CORE)
        gi = off + r
        sL = (s_cl[gi] - off) % N
        eL = (e_cl[gi] - off) % N
        wrap = nonempty[gi] & (sL > eL)
        ne = nonempty[gi]
        # interval list per row in local coords
        i1s = np.where(ne, np.where(wrap, 0, sL), 1)
        i1e = np.where(ne, eL, 0)
        i1v = ne.copy()
        i2s = np.where(wrap, sL, 1)
        i2e = np.where(wrap, np.int64(N - 1), 0)
        i2v = wrap.copy()
        per_core.append((off, i1s, i1e, i1v, i2s, i2e, i2v))
        for t in range(T):
            rt = slice(t * 128, (t + 1) * 128)
            for ss, ee, vv in ((i1s[rt], i1e[rt], i1v[rt]),
                               (i2s[rt], i2e[rt], i2v[rt])):
                ok = vv & (ss <= ee)
                if not ok.any():
                    continue
                for lo, hi in zip(ss[ok] // BLK, ee[ok] // BLK):
                    for bb in range(int(lo), int(hi) + 1):
                        act_sig[t].add(bb)
                        corr_sig[t].add(bb)
            corr_sig[t].add(t // 4)  # diagonal block always corrected

    corr_sig = tuple(tuple(sorted(s)) for s in corr_sig)
    act_sig = tuple(tuple(sorted(s)) for s in act_sig)

    corr_list = [(t, b) for t in range(T) for b in corr_sig[t]]
    act_list = [(t, b) for t in range(T) for b in act_sig[t]]
    n_corr, n_act = len(corr_list), len(act_list)

    # ---- per-core input maps ----
    in_maps = []
    for c in range(NCORES):
        off, i1s, i1e, i1v, i2s, i2e, i2v = per_core[c]
        # rotated fp8 matrix, layout [128, KCH, N]
        rolled = np.roll(zn8, -off, axis=0)              # [N, D]
        zn8_c = np.ascontiguousarray(
            rolled.T.reshape(KCH, 128, N).transpose(1, 0, 2))

        r = np.arange(ROWS_PER_CORE)
        gi = off + r

        scal_c = np.zeros((T, 128, 2), dtype=np.float32)
        flat = scal_c.reshape(ROWS_PER_CORE, 2)
        flat[:, 0] = -2.0 * nrm[gi] / SC2
        flat[:, 1] = sq[gi]

        # masks per corr entry: [n_corr, 128, 2, BLK] (mcorr, mpos)
        masks_c = np.zeros((max(n_corr, 1), 128, 2, BLK), dtype=np.float32)
        for idx, (t, b) in enumerate(corr_list):
            rt = np.arange(t * 128, (t + 1) * 128)
            cols = np.arange(b * BLK, (b + 1) * BLK)[None, :]
            inr = np.zeros((128, BLK), dtype=bool)
            for ss, ee, vv in ((i1s[rt], i1e[rt], i1v[rt]),
                               (i2s[rt], i2e[rt], i2v[rt])):
                inr |= vv[:, None] & (cols >= ss[:, None]) & \
                       (cols <= ee[:, None])
            diag = cols == rt[:, None]
            masks_c[idx, :, 0, :] = (inr | diag).astype(np.float32)
            masks_c[idx, :, 1, :] = (inr & ~diag).astype(np.float32)
        masks_c = masks_c.astype(ml_dtypes.bfloat16)

        # bc per act entry: [n_act, 128, BLK] (nrm_j broadcast; sq_j = nrm^2
        # is recomputed on-device inside the fused d2 op)
        nrm_rot = np.roll(nrm, -off)
        bc_c = np.zeros((max(n_act, 1), 128, BLK), dtype=np.float32)
        for idx, (t, b) in enumerate(act_list):
            bc_c[idx] = nrm_rot[b * BLK:(b + 1) * BLK][None, :]

        in_maps.append({
            "zn8": zn8_c, "masks": masks_c, "bc": bc_c, "scal": scal_c,
        })

    return in_maps, corr_sig, act_sig


# ---------------------------------------------------------------------------
# host-side finalize
# ---------------------------------------------------------------------------
def _host_finalize(pos_dev, neg_dev, starts, ends, M):
    s_arr = np.asarray(starts).astype(np.int64)[:M]
    e_arr = np.asarray(ends).astype(np.int64)[:M]
    i_arr = np.arange(M, dtype=np.int64)

    lo = np.maximum(s_arr, 0)
    hi = np.minimum(e_arr, N - 1)
    cnt_in = np.maximum(0, hi - lo + 1)
    in_i = ((i_arr >= s_arr) & (i_arr <= e_arr)).astype(np.int64)
    pos_cnt = cnt_in - in_i
    neg_cnt = N - cnt_in + in_i

    diag_term = (1.0 - M_NEG_SIM) ** 2  # exact j==i ortho entry
    pos_sum = pos_dev[:M].astype(np.float64)
    neg_sum = neg_dev[:M].astype(np.float64) + diag_term

    pos_pull = pos_sum / np.maximum(pos_cnt, 1)
    ortho = neg_sum / np.maximum(neg_cnt, 1)
    valid = (pos_cnt > 0) & (neg_cnt > 0)
    per_row = np.where(valid, pos_pull + LAM_NEG * ortho, 0.0)
    cnt = int(valid.sum())
    if cnt > 0:
        return np.float32(per_row.sum() / cnt)
    return np.float32(0.0)


# ---------------------------------------------------------------------------
# NTFF trace hook (profiling only; inert when KTRACE is unset)
# ---------------------------------------------------------------------------
def _install_trace_hook():
    import types
    try:
        import antenv
        if "antenv.axon_hooks" not in sys.modules:
            mod = types.ModuleType("antenv.axon_hooks")
            state = {"hook": None}
            mod.set_axon_ntff_profile_hook = \
                lambda h: state.__setitem__("hook", h)
            mod.get_axon_ntff_profile_hook = lambda: state["hook"]
            sys.modules["antenv.axon_hooks"] = mod
            antenv.axon_hooks = mod
        from antenv.axon_hooks import (
            get_axon_ntff_profile_hook, set_axon_ntff_profile_hook,
        )
        if get_axon_ntff_profile_hook() is None:
            from trn_agent_boot.trn_boot import _ntff_profile_via_ctypes
            set_axon_ntff_profile_hook(
                _ntff_profile_via_ctypes("/opt/axon/libaxon_pjrt.so"))
        import concourse.bass_utils as bu
        if not getattr(bu.upload_artifacts, "_stubbed", False):
            def _noop_upload(tmpdir):
                return tmpdir
            _noop_upload._stubbed = True
            bu.upload_artifacts = _noop_upload
        return True
    except Exception:
        return False


# ---------------------------------------------------------------------------
# entry point
# ---------------------------------------------------------------------------
def kernel(codebook, starts, ends, max_i):
    global last_exec_time_ns, _last_run

    codebook = np.asarray(codebook)
    assert codebook.shape == (N, D), codebook.shape
    M = min(N, int(max_i) + 1)

    in_maps, corr_sig, act_sig = _prepare_inputs(codebook, starts, ends)

    key = (corr_sig, act_sig)
    if key not in _programs:
        _programs[key] = _build_program(corr_sig, act_sig)
    nc = _programs[key]

    trace = bool(os.environ.get("KTRACE"))
    if trace:
        trace = _install_trace_hook()
    res = run_bass_kernel_spmd(
        nc, in_maps, core_ids=list(range(NCORES)), trace=trace)
    last_exec_time_ns = res.exec_time_ns
    _last_run = res

    pos_dev = np.empty(N, dtype=np.float64)
    neg_dev = np.empty(N, dtype=np.float64)
    for c in range(NCORES):
        s = res.results[c]["sums"].astype(np.float64)  # (T, 128, 3)
        off = c * ROWS_PER_CORE
        pos_dev[off:off + ROWS_PER_CORE] = s[..., 0].reshape(-1)
        neg_dev[off:off + ROWS_PER_CORE] = \
            (s[..., 1] - s[..., 2]).reshape(-1) / SC4

    return np.asarray(_host_finalize(pos_dev, neg_dev, starts, ends, M))
